# revision 9
# baseline (speedup 1.0000x reference)
"""BasinCoupledAttention Trainium2 kernel, v2 (restructured schedule).

Full inputs -> full output. Sharding: 2-way data parallel over batch x
4-way tensor parallel over heads (4 heads / core, 8 cores total).

Per-core device program (b = core//4, g = core%4, heads 4g..4g+3):
  - DMA issue order == consumption order: wq, wk, bq, bk, xt chunk 0,
    wv, bv, tri, xt chunks 1-3, wo.  proj(nb) consumes xt chunk nb.
  - projections stream per 512-column chunk nb: qT/kT = W^T x^T in
    [d', s] fp16 (basin gate and 1/sqrt(dh) folded into Wq/bq on host),
    v1 = [x Wv + bv | 1] in [s, h, 65] fp16.
  - attention per (t = head pair, ib = 512-query chunk), block-causal
    over jb key blocks of 128: per head S^T = kT^T qT into its own
    1-bank PSUM tile (pairs row-tile on HW), per-head exp on ACT (no
    max subtraction; |scores| <= ~6) into fp16 a_t, fp16 tri-mask
    multiply on DVE for diagonal squares, [out^T; rowsum] accumulated
    via v1 fp16 matmuls into [65, 512] PSUM, normalize = reciprocal
    (DVE) + partition broadcast (Pool) + multiply (DVE) into oT fp16.
  - qk(nb+1) / v(nb+1) / outp(ib-1) groups are injected at per-jb
    spread points inside attn(ib) so the in-order PE queue always has
    work whose DMA dependencies have already arrived.
  - outp per 128-row chunk: O^T' Wo accumulated over t (stationary
    operand reused across both n halves), evicted fp16 on DVE (ACT
    helps at the tail), DMA'd out fp16.
Host sums the 4 per-core fp16 partials for each batch in fp32, adds bo.
"""

import sys

if "/opt/trn_rl_repo" not in sys.path:
    sys.path.insert(0, "/opt/trn_rl_repo")

import numpy as np

D = 1024          # d_model
S = 2048          # sequence
B = 2             # batch
HL = 4            # heads per core
DL = 256          # d' columns per core (HL * 64)
DH = 64           # head dim
N_CORES = 8

_CACHE = {}


def _build_program(reps=1, stop_after="all", loop_only=None,
                   tiny_out=False, bcast="pool", exp_copy=False,
                   norm=True, notri=False, trimm=False,
                   stagger=False, exp_split=True):
    import concourse.bacc as bacc
    import concourse.mybir as mybir
    import concourse.tile as tile

    f32 = mybir.dt.float32
    f16 = mybir.dt.float16
    Exp = (mybir.ActivationFunctionType.Copy if exp_copy
           else mybir.ActivationFunctionType.Exp)

    nc = bacc.Bacc("TRN2", target_bir_lowering=False, debug=False)

    # host pre-arranges everything so each DMA is contiguous on both
    # sides: xt[q-chunk][p][kd][s'], w*[p][kd][c], wo[p][t][c], bqk[p][g]
    xt_d = nc.dram_tensor("xt", [4 * 128, 8 * 512], f16,
                          kind="ExternalInput").ap()
    wq_d = nc.dram_tensor("wq", [128, 8 * DL], f16, kind="ExternalInput").ap()
    wk_d = nc.dram_tensor("wk", [128, 8 * DL], f16, kind="ExternalInput").ap()
    wv_d = nc.dram_tensor("wv", [128, 8 * DL], f16, kind="ExternalInput").ap()
    wo_d = nc.dram_tensor("wo", [128, 2 * D], f16, kind="ExternalInput").ap()
    bqk_d = nc.dram_tensor("bqk", [128, 4], f32, kind="ExternalInput").ap()
    bv_d = nc.dram_tensor("bv", [1, HL * 64], f32, kind="ExternalInput").ap()
    # msk = [I | M]: 128x128 identity and -1000 * strict-lower-triangle
    tri_d = nc.dram_tensor("tri", [128, 384], f16, kind="ExternalInput").ap()
    if tiny_out:
        out_d = nc.dram_tensor("out_big", [S, D], f16).ap()
        out_small = nc.dram_tensor("out", [128, D], f16,
                                   kind="ExternalOutput").ap()
    else:
        out_d = nc.dram_tensor("out", [S, D], f16, kind="ExternalOutput").ap()
        out_small = None

    with tile.TileContext(nc) as tc:
        with (
            tc.tile_pool(name="persist", bufs=1) as pp,
            tc.tile_pool(name="work", bufs=4) as wp,
            tc.tile_pool(name="ps_a", bufs=5, space="PSUM") as ps_a,
            tc.tile_pool(name="ps_av", bufs=3, space="PSUM") as ps_av,
        ):
            # ---------- persistent SBUF tiles -------------------------
            # flat DMA-landing tiles; consumers use rearranged views
            # xt: [p, q-chunk, kd, s'], w*: [p, kd, c], wo: [p, t, c]
            xt_sb = pp.tile([128, 4 * 8 * 512], f16, name="xt", tag="xt")
            wq_sb = pp.tile([128, 8 * DL], f16, name="wq", tag="wq")
            wk_sb = pp.tile([128, 8 * DL], f16, name="wk", tag="wk")
            wv_sb = pp.tile([128, 8 * DL], f16, name="wv", tag="wv")
            wo_sb = pp.tile([128, 2 * D], f16, name="wo", tag="wo")
            xt_v = xt_sb.rearrange("p (q k s) -> p q k s", q=4, k=8)
            wq_v = wq_sb.rearrange("p (k c) -> p k c", k=8)
            wk_v = wk_sb.rearrange("p (k c) -> p k c", k=8)
            wv_v = wv_sb.rearrange("p (k c) -> p k c", k=8)
            wo_v = wo_sb.rearrange("p (t c) -> p t c", t=2)
            # [bq t0 | bq t1 | bk t0 | bk t1] as 4 per-partition columns
            bqk_sb = pp.tile([128, 4], f32, name="bqk", tag="bqk")
            bv_sb = pp.tile([128, HL * 64], f32, name="bv", tag="bv")
            tri_sb = pp.tile([128, 384], f16, name="tri", tag="tri")
            qT_sb = [pp.tile([128, S], f16, name=f"qT{t}", tag=f"qT{t}")
                     for t in range(2)]
            kT_sb = [pp.tile([128, S], f16, name=f"kT{t}", tag=f"kT{t}")
                     for t in range(2)]
            v1_sb = [pp.tile([128, HL, 65], f16, name=f"v1_{s_}",
                             tag=f"v1_{s_}") for s_ in range(16)]
            oT_sb = [pp.tile([128, S], f16, name=f"oT{t}", tag=f"oT{t}")
                     for t in range(2)]
            bv_r = bv_sb.rearrange("p (h c) -> p h c", h=HL)

            # ones column of v1 is input-independent: set once, pre-loop
            for s_ in range(16):
                nc.gpsimd.memset(v1_sb[s_][:, :, 64:65], 1.0)
            ones64 = pp.tile([1, 64], f16, name="ones64", tag="ones64")
            nc.gpsimd.memset(ones64, 1.0)
            warm_rhs = pp.tile([1, 512], f16, name="warm", tag="warm")
            nc.gpsimd.memset(warm_rhs, 1.0)

            # ---------- per-iteration phases --------------------------
            def ph_dma():
                nc.sync.dma_start(out=wq_sb[:, 0:1024], in_=wq_d[:, 0:1024])
                nc.sync.dma_start(out=xt_sb[:, 0:2048],
                                  in_=xt_d[0:128, 0:2048])
                nc.sync.dma_start(out=wq_sb[:, 1024:2048],
                                  in_=wq_d[:, 1024:2048])
                nc.sync.dma_start(out=xt_sb[:, 2048:4096],
                                  in_=xt_d[0:128, 2048:4096])
                nc.sync.dma_start(out=wk_sb, in_=wk_d)
                nc.sync.dma_start(out=bqk_sb, in_=bqk_d)
                nc.sync.dma_start(out=wv_sb, in_=wv_d)
                nc.sync.dma_start(out=bv_sb,
                                  in_=bv_d.to_broadcast([128, HL * 64]))
                nc.sync.dma_start(out=tri_sb, in_=tri_d)
                for q in range(1, 4):
                    nc.sync.dma_start(out=xt_sb[:, q * 4096:(q + 1) * 4096],
                                      in_=xt_d[q * 128:(q + 1) * 128, :])
                nc.sync.dma_start(out=wo_sb, in_=wo_d)

            ps_bufs = 5 if exp_split else 2

            def warm_pe(n):
                # keep the PE busy (HAM warm) across DMA-wait windows;
                # writes a scratch PSUM region nothing reads
                ps_w = ps_av.tile([64, 512], f32, name="ps_w", tag="av",
                                  bufs=3 if exp_split else 2)
                for _ in range(n):
                    nc.tensor.matmul(ps_w, lhsT=ones64, rhs=warm_rhs,
                                     start=True, stop=True,
                                     skip_group_check=True)

            def qk_group(t, kind, nb, midfill=None):
                w_sb, bcol, dst = ((wq_v, t, qT_sb) if kind == "q"
                                   else (wk_v, 2 + t, kT_sb))

                def g():
                    ps_p = ps_a.tile([128, 512], f32, name="ps_p", tag="ps", bufs=ps_bufs)
                    for kd in range(8):
                        if kd == 4 and midfill is not None:
                            midfill()
                        nc.tensor.matmul(
                            ps_p,
                            lhsT=w_sb[:, kd, t * 128:(t + 1) * 128],
                            rhs=xt_v[:, nb, kd, :],
                            start=(kd == 0), stop=(kd == 7),
                        )
                    nc.vector.tensor_scalar_add(
                        out=dst[t][:, nb * 512:(nb + 1) * 512],
                        in0=ps_p, scalar1=bqk_sb[:, bcol:bcol + 1],
                    )
                return g

            def v_group(sc):
                def g():
                    ps_v = ps_a.tile([128, DL], f32, name="ps_v", tag="ps", bufs=ps_bufs)
                    for kd in range(8):
                        nc.tensor.matmul(
                            ps_v,
                            lhsT=xt_v[:, sc // 4, kd,
                                      (sc % 4) * 128:(sc % 4 + 1) * 128],
                            rhs=wv_v[:, kd, :],
                            start=(kd == 0), stop=(kd == 7),
                        )
                    nc.vector.tensor_tensor(
                        out=v1_sb[sc][:, :, 0:64],
                        in0=ps_v.rearrange("p (h d) -> p h d", h=HL),
                        in1=bv_r,
                        op=mybir.AluOpType.add,
                    )
                return g

            def outp_group(sc, act_evict=False):
                def g():
                    o_sb = wp.tile([128, D], f16, name="o_sb", tag="osb",
                                   bufs=5)
                    # t outer so each oT slice (stationary operand) is
                    # loaded once for both n halves
                    ps_f = [ps_a.tile([128, 512], f32, name="ps_f",
                                      tag="ps", bufs=ps_bufs)
                            for _ in range(2)]
                    for t in range(2):
                        for n in range(2):
                            nc.tensor.matmul(
                                ps_f[n],
                                lhsT=oT_sb[t][:, sc * 128:(sc + 1) * 128],
                                rhs=wo_v[:, t, n * 512:(n + 1) * 512],
                                start=(t == 0), stop=(t == 1),
                                skip_group_check=True,
                            )
                    for n in range(2):
                        if act_evict and n == 1:
                            nc.scalar.copy(
                                o_sb[:, n * 512:(n + 1) * 512], ps_f[n])
                        else:
                            nc.vector.tensor_copy(
                                o_sb[:, n * 512:(n + 1) * 512], ps_f[n])
                    nc.sync.dma_start(
                        out=out_d[sc * 128:(sc + 1) * 128, :], in_=o_sb)
                    if out_small is not None and sc == 15:
                        nc.sync.dma_start(out=out_small, in_=o_sb)
                return g

            def attn_block(t, ib, inject):
                njb = 4 * ib + 4
                avb = 3 if exp_split else 2
                ps_o = {
                    0: ps_av.tile([65, 512], f32, name="ps_o0", tag="av",
                                  bufs=avb),
                    1: ps_av.tile([65, 512], f32, name="ps_o1", tag="av",
                                  bufs=avb),
                }
                pend = []

                def flush_av(limit):
                    while len(pend) > limit:
                        h, jb, off, a_t = pend.pop(0)
                        nc.tensor.matmul(
                            ps_o[h][:, off:512],
                            lhsT=v1_sb[jb][:, 2 * t + h, :],
                            rhs=a_t[:, 512 * h + off:512 * (h + 1)],
                            start=(jb == 0), stop=(jb == njb - 1),
                            skip_group_check=True,
                        )

                inject = list(inject)
                ninj = len(inject)
                for jb in range(njb):
                    diag = jb >= 4 * ib
                    off = jb * 128 - ib * 512 if diag else 0
                    a_t = wp.tile([128, 1024], f16, name="a_t", tag="a",
                                  bufs=10)
                    if exp_split:
                        for h in (0, 1):
                            bp = h * 64
                            ps_s = ps_a.tile([128, 512], f32, name="ps_s",
                                             tag="ps", bufs=ps_bufs)
                            nc.tensor.matmul(
                                ps_s[:, off:512],
                                lhsT=kT_sb[t][bp:bp + 64,
                                              jb * 128:(jb + 1) * 128],
                                rhs=qT_sb[t][bp:bp + 64,
                                             ib * 512 + off:(ib + 1) * 512],
                                start=True, stop=True,
                            )
                            if diag and trimm and not notri:
                                # accumulate -1000 on the strictly-lower
                                # triangle of the diag square: I.T @ M = M
                                nc.tensor.matmul(
                                    ps_s[:, off:off + 128],
                                    lhsT=tri_sb[:, 0:128],
                                    rhs=tri_sb[:, 128:256],
                                    start=False, stop=True,
                                    skip_group_check=True,
                                )
                            nc.scalar.activation(
                                a_t[:, 512 * h + off:512 * (h + 1)],
                                ps_s[:, off:512], Exp)
                            if diag and not trimm and not notri:
                                nc.vector.tensor_tensor(
                                    out=a_t[:, 512 * h + off:
                                            512 * h + off + 128],
                                    in0=a_t[:, 512 * h + off:
                                            512 * h + off + 128],
                                    in1=tri_sb[:, 256:384],
                                    op=mybir.AluOpType.mult,
                                )
                            pend.append((h, jb, off, a_t))
                    else:
                        ps_s = ps_a.tile([128, 1024], f32, name="ps_s2",
                                         tag="ps2", bufs=2)
                        for h in (0, 1):
                            bp = h * 64
                            nc.tensor.matmul(
                                ps_s[:, 512 * h + off:512 * (h + 1)],
                                lhsT=kT_sb[t][bp:bp + 64,
                                              jb * 128:(jb + 1) * 128],
                                rhs=qT_sb[t][bp:bp + 64,
                                             ib * 512 + off:(ib + 1) * 512],
                                start=True, stop=True,
                            )
                        if diag:
                            a_r = a_t.rearrange("p (h q) -> p h q", h=2)
                            s_r = ps_s.rearrange("p (h q) -> p h q", h=2)
                            nc.scalar.activation(a_r[:, :, off:512],
                                                 s_r[:, :, off:512], Exp)
                            for h in (0, 1):
                                if notri:
                                    continue
                                nc.vector.tensor_tensor(
                                    out=a_t[:, 512 * h + off:
                                            512 * h + off + 128],
                                    in0=a_t[:, 512 * h + off:
                                            512 * h + off + 128],
                                    in1=tri_sb[:, 256:384],
                                    op=mybir.AluOpType.mult,
                                )
                        else:
                            nc.scalar.activation(a_t, ps_s, Exp)
                        pend.append((0, jb, off, a_t))
                        pend.append((1, jb, off, a_t))
                    flush_av(4)
                    want = -((-ninj * (jb + 1)) // njb)
                    while ninj - len(inject) < want:
                        inject.pop(0)()
                # drain h0's AV chain first so its normalize overlaps
                # h1's remaining AV matmuls
                recip, rb = {}, {}

                def norm_head(h):
                    recip[h] = wp.tile([1, 512], f32, name="recip", tag="rc",
                                       bufs=4)
                    nc.vector.reciprocal(recip[h], ps_o[h][64:65, :])
                    rb[h] = wp.tile([64, 512], f32, name="rb", tag="rb",
                                    bufs=4)
                    nc.gpsimd.partition_broadcast(rb[h], recip[h])

                for h in (0, 1):
                    rest = [e for e in pend if e[0] == h]
                    for e in rest:
                        pend.remove(e)
                    for _, jb, off, a_t in rest:
                        nc.tensor.matmul(
                            ps_o[h][:, off:512],
                            lhsT=v1_sb[jb][:, 2 * t + h, :],
                            rhs=a_t[:, 512 * h + off:512 * (h + 1)],
                            start=(jb == 0), stop=(jb == njb - 1),
                            skip_group_check=True,
                        )
                    if norm:
                        norm_head(h)
                while inject:
                    inject.pop(0)()
                if not norm:
                    return
                for h in (0, 1):
                    bp = h * 64
                    nc.vector.tensor_tensor(
                        out=oT_sb[t][bp:bp + 64, ib * 512:(ib + 1) * 512],
                        in0=ps_o[h][0:64, :], in1=rb[h],
                        op=mybir.AluOpType.mult,
                    )

            def body():
                warm_pe(12)
                ph_dma()
                if stop_after == "dma":
                    return
                if stop_after == "proj":
                    for nb_ in range(4):
                        for t_, kind in ((0, "q"), (0, "k"),
                                         (1, "q"), (1, "k")):
                            qk_group(t_, kind, nb_)()
                        for sc in range(4 * nb_, 4 * nb_ + 4):
                            v_group(sc)()
                    return
                # head: just enough for attn(0, 0)
                qk_group(0, "q", 0)()
                qk_group(0, "k", 0)()
                for sc in range(4):
                    v_group(sc)()
                # injected fillers per attn segment, ordered by DMA
                # arrival and consumer position
                inj = {
                    (0, 0): [qk_group(1, "q", 0), qk_group(1, "k", 0)],
                    (1, 0): [qk_group(0, "q", 1), qk_group(0, "k", 1),
                             v_group(4), v_group(5), v_group(6), v_group(7),
                             qk_group(1, "q", 1), qk_group(1, "k", 1)],
                    (0, 1): [qk_group(0, "q", 2), qk_group(0, "k", 2)],
                    (1, 1): [v_group(8), v_group(9), v_group(10),
                             v_group(11), qk_group(1, "q", 2),
                             qk_group(1, "k", 2),
                             outp_group(0), outp_group(1)],
                    (0, 2): [outp_group(2), outp_group(3),
                             qk_group(0, "q", 3), qk_group(0, "k", 3)],
                    (1, 2): [v_group(12), v_group(13), v_group(14),
                             v_group(15), qk_group(1, "q", 3),
                             qk_group(1, "k", 3),
                             outp_group(4), outp_group(5)],
                    (0, 3): [outp_group(6), outp_group(7),
                             outp_group(8), outp_group(9)],
                    (1, 3): [outp_group(10), outp_group(11)],
                }
                for ib in range(4):
                    for t_ in range(2):
                        attn_block(t_, ib, inj[(t_, ib)])
                for sc in range(12, 16):
                    outp_group(sc, act_evict=True)()

            if reps == 1:
                body()
            else:
                with tc.For_i(0, reps, 1, staggered_reset=stagger,
                              hint_engines=(mybir.EngineType.PE,
                                            mybir.EngineType.Activation,
                                            mybir.EngineType.DVE)):
                    body()

    nc.compile()
    return nc


def _prepare_in_maps(inputs):
    x = np.asarray(inputs["x"], np.float32)
    basin = np.asarray(inputs["basin"], np.float32)
    Wq = np.asarray(inputs["Wq"], np.float32)
    bq = np.asarray(inputs["bq"], np.float32)
    Wk = np.asarray(inputs["Wk"], np.float32)
    bk = np.asarray(inputs["bk"], np.float32)
    Wv = np.asarray(inputs["Wv"], np.float32)
    bv = np.asarray(inputs["bv"], np.float32)
    Wo = np.asarray(inputs["Wo"], np.float32)
    Wb = np.asarray(inputs["Wb"], np.float32)
    bb = np.asarray(inputs["bb"], np.float32)

    gate = 1.0 / (1.0 + np.exp(-(basin @ Wb + bb)))          # [16]
    scale = (gate / np.sqrt(float(DH))).astype(np.float32)   # [16]
    colscale = np.repeat(scale, DH)                          # [1024]
    Wq_s = (Wq * colscale[None, :]).astype(np.float32)
    bq_s = (bq * colscale).astype(np.float32)

    # xt host layout: [q-chunk*128p, kd*512s]: (q*128+p, k*512+s) = xT[k*128+p, q*512+s]
    xt_all = []
    for b in range(B):
        xt_b = x[b].T.astype(np.float16)          # [1024, 2048]
        A = xt_b.reshape(8, 128, 4, 512).transpose(2, 1, 0, 3)
        xt_all.append(np.ascontiguousarray(A.reshape(4 * 128, 8 * 512)))
    eye = np.eye(128, dtype=np.float16)
    msk = (np.tril(np.ones((128, 128), np.float32), -1) *
           -1000.0).astype(np.float16)
    triu = np.triu(np.ones((128, 128), np.float16))
    tri2 = np.ascontiguousarray(np.concatenate([eye, msk, triu], axis=1))

    in_maps = []
    for c in range(N_CORES):
        b, g = divmod(c, 4)
        sl = slice(g * DL, (g + 1) * DL)
        def stack_w(W):
            # [1024, 256] -> [p, kd*256] with element (p, k*256+c) = W[k*128+p, c]
            return np.ascontiguousarray(
                W.reshape(8, 128, DL).transpose(1, 0, 2).reshape(128, 8 * DL)
            ).astype(np.float16)

        in_maps.append({
            "xt": xt_all[b],
            "wq": stack_w(Wq_s[:, sl]),
            "wk": stack_w(Wk[:, sl]),
            "wv": stack_w(Wv[:, sl]),
            "wo": np.ascontiguousarray(
                Wo[sl, :].reshape(2, 128, D).transpose(1, 0, 2)
                .reshape(128, 2 * D)).astype(np.float16),
            "bqk": np.ascontiguousarray(
                np.concatenate([bq_s[sl], bk[sl]]).reshape(4, 128).T),
            "bv": np.ascontiguousarray(bv[sl]).reshape(1, DL),
            "tri": tri2,
        })
    return in_maps


def _run(inputs, trace=False):
    from concourse.bass_utils import run_bass_kernel_spmd

    nc = _CACHE.get("nc")
    if nc is None:
        nc = _build_program()
        _CACHE["nc"] = nc
    in_maps = _prepare_in_maps(inputs)
    res = run_bass_kernel_spmd(nc, in_maps, core_ids=list(range(N_CORES)),
                               trace=trace)
    bo = np.asarray(inputs["bo"], np.float32)
    out = np.zeros((B, S, D), np.float32)
    for c in range(N_CORES):
        out[c // 4] += res.results[c]["out"].astype(np.float32)
    out += bo[None, None, :]
    return out, res


def kernel(**inputs):
    out, _ = _run(inputs, trace=False)
    return out


# revision 10
# speedup vs baseline: 1.0218x; 1.0218x over previous
"""BasinCoupledAttention Trainium2 kernel, v2 (restructured schedule).

Full inputs -> full output. Sharding: 2-way data parallel over batch x
4-way tensor parallel over heads (4 heads / core, 8 cores total).

Per-core device program (b = core//4, g = core%4, heads 4g..4g+3):
  - DMA issue order == consumption order: wq, wk, bq, bk, xt chunk 0,
    wv, bv, tri, xt chunks 1-3, wo.  proj(nb) consumes xt chunk nb.
  - projections stream per 512-column chunk nb: qT/kT = W^T x^T in
    [d', s] fp16 (basin gate and 1/sqrt(dh) folded into Wq/bq on host),
    v1 = [x Wv + bv | 1] in [s, h, 65] fp16.
  - attention per (t = head pair, ib = 512-query chunk), block-causal
    over jb key blocks of 128: per head S^T = kT^T qT into its own
    1-bank PSUM tile (pairs row-tile on HW), per-head exp on ACT (no
    max subtraction; |scores| <= ~6) into fp16 a_t, fp16 tri-mask
    multiply on DVE for diagonal squares, [out^T; rowsum] accumulated
    via v1 fp16 matmuls into [65, 512] PSUM, normalize = reciprocal
    (DVE) + partition broadcast (Pool) + multiply (DVE) into oT fp16.
  - qk(nb+1) / v(nb+1) / outp(ib-1) groups are injected at per-jb
    spread points inside attn(ib) so the in-order PE queue always has
    work whose DMA dependencies have already arrived.
  - outp per 128-row chunk: O^T' Wo accumulated over t (stationary
    operand reused across both n halves), evicted fp16 on DVE (ACT
    helps at the tail), DMA'd out fp16.
Host sums the 4 per-core fp16 partials for each batch in fp32, adds bo.
"""

import sys

if "/opt/trn_rl_repo" not in sys.path:
    sys.path.insert(0, "/opt/trn_rl_repo")

import numpy as np

D = 1024          # d_model
S = 2048          # sequence
B = 2             # batch
HL = 4            # heads per core
DL = 256          # d' columns per core (HL * 64)
DH = 64           # head dim
N_CORES = 8

_CACHE = {}


def _build_program(reps=1, stop_after="all", loop_only=None,
                   tiny_out=False, bcast="pool", exp_copy=False,
                   norm=True, notri=False, trimm=False,
                   stagger=False, exp_split=True):
    import concourse.bacc as bacc
    import concourse.mybir as mybir
    import concourse.tile as tile

    f32 = mybir.dt.float32
    f16 = mybir.dt.float16
    Exp = (mybir.ActivationFunctionType.Copy if exp_copy
           else mybir.ActivationFunctionType.Exp)

    nc = bacc.Bacc("TRN2", target_bir_lowering=False, debug=False)

    # host pre-arranges everything so each DMA is contiguous on both
    # sides: xt[q-chunk][p][kd][s'], w*[p][kd][c], wo[p][t][c], bqk[p][g]
    xt_d = nc.dram_tensor("xt", [4 * 128, 8 * 512], f16,
                          kind="ExternalInput").ap()
    wq_d = nc.dram_tensor("wq", [128, 8 * DL], f16, kind="ExternalInput").ap()
    wk_d = nc.dram_tensor("wk", [128, 8 * DL], f16, kind="ExternalInput").ap()
    wv_d = nc.dram_tensor("wv", [128, 8 * DL], f16, kind="ExternalInput").ap()
    wo_d = nc.dram_tensor("wo", [128, 2 * D], f16, kind="ExternalInput").ap()
    bqk_d = nc.dram_tensor("bqk", [128, 4], f32, kind="ExternalInput").ap()
    bv_d = nc.dram_tensor("bv", [1, HL * 64], f32, kind="ExternalInput").ap()
    # msk = [I | M]: 128x128 identity and -1000 * strict-lower-triangle
    tri_d = nc.dram_tensor("tri", [128, 384], f16, kind="ExternalInput").ap()
    if tiny_out:
        out_d = nc.dram_tensor("out_big", [S, D], f16).ap()
        out_small = nc.dram_tensor("out", [128, D], f16,
                                   kind="ExternalOutput").ap()
    else:
        out_d = nc.dram_tensor("out", [S, D], f16, kind="ExternalOutput").ap()
        out_small = None

    with tile.TileContext(nc) as tc:
        with (
            tc.tile_pool(name="persist", bufs=1) as pp,
            tc.tile_pool(name="work", bufs=4) as wp,
            tc.tile_pool(name="ps_a", bufs=5, space="PSUM") as ps_a,
            tc.tile_pool(name="ps_av", bufs=3, space="PSUM") as ps_av,
        ):
            # ---------- persistent SBUF tiles -------------------------
            # flat DMA-landing tiles; consumers use rearranged views
            # xt: [p, q-chunk, kd, s'], w*: [p, kd, c], wo: [p, t, c]
            xt_sb = pp.tile([128, 4 * 8 * 512], f16, name="xt", tag="xt")
            wq_sb = pp.tile([128, 8 * DL], f16, name="wq", tag="wq")
            wk_sb = pp.tile([128, 8 * DL], f16, name="wk", tag="wk")
            wv_sb = pp.tile([128, 8 * DL], f16, name="wv", tag="wv")
            wo_sb = pp.tile([128, 2 * D], f16, name="wo", tag="wo")
            xt_v = xt_sb.rearrange("p (q k s) -> p q k s", q=4, k=8)
            wq_v = wq_sb.rearrange("p (k c) -> p k c", k=8)
            wk_v = wk_sb.rearrange("p (k c) -> p k c", k=8)
            wv_v = wv_sb.rearrange("p (k c) -> p k c", k=8)
            wo_v = wo_sb.rearrange("p (t c) -> p t c", t=2)
            # [bq t0 | bq t1 | bk t0 | bk t1] as 4 per-partition columns
            bqk_sb = pp.tile([128, 4], f32, name="bqk", tag="bqk")
            bv_sb = pp.tile([128, HL * 64], f32, name="bv", tag="bv")
            tri_sb = pp.tile([128, 384], f16, name="tri", tag="tri")
            qT_sb = [pp.tile([128, S], f16, name=f"qT{t}", tag=f"qT{t}")
                     for t in range(2)]
            kT_sb = [pp.tile([128, S], f16, name=f"kT{t}", tag=f"kT{t}")
                     for t in range(2)]
            v1_sb = [pp.tile([128, HL, 65], f16, name=f"v1_{s_}",
                             tag=f"v1_{s_}") for s_ in range(16)]
            oT_sb = [pp.tile([128, S], f16, name=f"oT{t}", tag=f"oT{t}")
                     for t in range(2)]
            bv_r = bv_sb.rearrange("p (h c) -> p h c", h=HL)

            # ones column of v1 is input-independent: set once, pre-loop
            for s_ in range(16):
                nc.gpsimd.memset(v1_sb[s_][:, :, 64:65], 1.0)
            ones64 = pp.tile([1, 64], f16, name="ones64", tag="ones64")
            nc.gpsimd.memset(ones64, 1.0)
            warm_rhs = pp.tile([1, 512], f16, name="warm", tag="warm")
            nc.gpsimd.memset(warm_rhs, 1.0)

            # ---------- per-iteration phases --------------------------
            def ph_dma():
                nc.sync.dma_start(out=wq_sb[:, 0:1024], in_=wq_d[:, 0:1024])
                nc.sync.dma_start(out=xt_sb[:, 0:2048],
                                  in_=xt_d[0:128, 0:2048])
                nc.sync.dma_start(out=wq_sb[:, 1024:2048],
                                  in_=wq_d[:, 1024:2048])
                nc.sync.dma_start(out=xt_sb[:, 2048:4096],
                                  in_=xt_d[0:128, 2048:4096])
                nc.sync.dma_start(out=wk_sb, in_=wk_d)
                nc.sync.dma_start(out=bqk_sb, in_=bqk_d)
                nc.sync.dma_start(out=wv_sb, in_=wv_d)
                nc.sync.dma_start(out=bv_sb,
                                  in_=bv_d.to_broadcast([128, HL * 64]))
                nc.sync.dma_start(out=tri_sb, in_=tri_d)
                for q in range(1, 4):
                    nc.sync.dma_start(out=xt_sb[:, q * 4096:(q + 1) * 4096],
                                      in_=xt_d[q * 128:(q + 1) * 128, :])
                nc.sync.dma_start(out=wo_sb, in_=wo_d)

            ps_bufs = 5 if exp_split else 2

            def warm_pe(n):
                # keep the PE busy (HAM warm) across DMA-wait windows;
                # writes a scratch PSUM region nothing reads
                ps_w = ps_av.tile([64, 512], f32, name="ps_w", tag="av",
                                  bufs=3 if exp_split else 2)
                for _ in range(n):
                    nc.tensor.matmul(ps_w, lhsT=ones64, rhs=warm_rhs,
                                     start=True, stop=True,
                                     skip_group_check=True)

            def qk_group(t, kind, nb, midfill=None):
                w_sb, bcol, dst = ((wq_v, t, qT_sb) if kind == "q"
                                   else (wk_v, 2 + t, kT_sb))

                def g():
                    ps_p = ps_a.tile([128, 512], f32, name="ps_p", tag="ps", bufs=ps_bufs)
                    for kd in range(8):
                        if kd == 4 and midfill is not None:
                            midfill()
                        nc.tensor.matmul(
                            ps_p,
                            lhsT=w_sb[:, kd, t * 128:(t + 1) * 128],
                            rhs=xt_v[:, nb, kd, :],
                            start=(kd == 0), stop=(kd == 7),
                        )
                    nc.vector.tensor_scalar_add(
                        out=dst[t][:, nb * 512:(nb + 1) * 512],
                        in0=ps_p, scalar1=bqk_sb[:, bcol:bcol + 1],
                    )
                return g

            def v_group(sc):
                def g():
                    ps_v = ps_a.tile([128, DL], f32, name="ps_v", tag="ps", bufs=ps_bufs)
                    for kd in range(8):
                        nc.tensor.matmul(
                            ps_v,
                            lhsT=xt_v[:, sc // 4, kd,
                                      (sc % 4) * 128:(sc % 4 + 1) * 128],
                            rhs=wv_v[:, kd, :],
                            start=(kd == 0), stop=(kd == 7),
                        )
                    nc.vector.tensor_tensor(
                        out=v1_sb[sc][:, :, 0:64],
                        in0=ps_v.rearrange("p (h d) -> p h d", h=HL),
                        in1=bv_r,
                        op=mybir.AluOpType.add,
                    )
                return g

            def outp_group(sc, act_evict=False):
                def g():
                    o_sb = wp.tile([128, D], f16, name="o_sb", tag="osb",
                                   bufs=5)
                    # t outer so each oT slice (stationary operand) is
                    # loaded once for both n halves
                    ps_f = [ps_a.tile([128, 512], f32, name="ps_f",
                                      tag="ps", bufs=ps_bufs)
                            for _ in range(2)]
                    for t in range(2):
                        for n in range(2):
                            nc.tensor.matmul(
                                ps_f[n],
                                lhsT=oT_sb[t][:, sc * 128:(sc + 1) * 128],
                                rhs=wo_v[:, t, n * 512:(n + 1) * 512],
                                start=(t == 0), stop=(t == 1),
                                skip_group_check=True,
                            )
                    for n in range(2):
                        if act_evict and n == 1:
                            nc.scalar.copy(
                                o_sb[:, n * 512:(n + 1) * 512], ps_f[n])
                        else:
                            nc.vector.tensor_copy(
                                o_sb[:, n * 512:(n + 1) * 512], ps_f[n])
                    nc.sync.dma_start(
                        out=out_d[sc * 128:(sc + 1) * 128, :], in_=o_sb)
                    if out_small is not None and sc == 15:
                        nc.sync.dma_start(out=out_small, in_=o_sb)
                return g

            def attn_block(t, ib, inject):
                njb = 4 * ib + 4
                avb = 3 if exp_split else 2
                ps_o = {
                    0: ps_av.tile([65, 512], f32, name="ps_o0", tag="av",
                                  bufs=avb),
                    1: ps_av.tile([65, 512], f32, name="ps_o1", tag="av",
                                  bufs=avb),
                }
                pend = []

                def flush_av(limit):
                    while len(pend) > limit:
                        h, jb, off, a_t = pend.pop(0)
                        nc.tensor.matmul(
                            ps_o[h][:, off:512],
                            lhsT=v1_sb[jb][:, 2 * t + h, :],
                            rhs=a_t[:, 512 * h + off:512 * (h + 1)],
                            start=(jb == 0), stop=(jb == njb - 1),
                            skip_group_check=True,
                        )

                inject = list(inject)
                ninj = len(inject)
                for jb in range(njb):
                    diag = jb >= 4 * ib
                    off = jb * 128 - ib * 512 if diag else 0
                    a_t = wp.tile([128, 1024], f16, name="a_t", tag="a",
                                  bufs=10)
                    if exp_split:
                        for h in (0, 1):
                            bp = h * 64
                            ps_s = ps_a.tile([128, 512], f32, name="ps_s",
                                             tag="ps", bufs=ps_bufs)
                            nc.tensor.matmul(
                                ps_s[:, off:512],
                                lhsT=kT_sb[t][bp:bp + 64,
                                              jb * 128:(jb + 1) * 128],
                                rhs=qT_sb[t][bp:bp + 64,
                                             ib * 512 + off:(ib + 1) * 512],
                                start=True, stop=True,
                            )
                            if diag and trimm and not notri:
                                # accumulate -1000 on the strictly-lower
                                # triangle of the diag square: I.T @ M = M
                                nc.tensor.matmul(
                                    ps_s[:, off:off + 128],
                                    lhsT=tri_sb[:, 0:128],
                                    rhs=tri_sb[:, 128:256],
                                    start=False, stop=True,
                                    skip_group_check=True,
                                )
                            nc.scalar.activation(
                                a_t[:, 512 * h + off:512 * (h + 1)],
                                ps_s[:, off:512], Exp)
                            if diag and not trimm and not notri:
                                nc.vector.tensor_tensor(
                                    out=a_t[:, 512 * h + off:
                                            512 * h + off + 128],
                                    in0=a_t[:, 512 * h + off:
                                            512 * h + off + 128],
                                    in1=tri_sb[:, 256:384],
                                    op=mybir.AluOpType.mult,
                                )
                            pend.append((h, jb, off, a_t))
                    else:
                        ps_s = ps_a.tile([128, 1024], f32, name="ps_s2",
                                         tag="ps2", bufs=2)
                        for h in (0, 1):
                            bp = h * 64
                            nc.tensor.matmul(
                                ps_s[:, 512 * h + off:512 * (h + 1)],
                                lhsT=kT_sb[t][bp:bp + 64,
                                              jb * 128:(jb + 1) * 128],
                                rhs=qT_sb[t][bp:bp + 64,
                                             ib * 512 + off:(ib + 1) * 512],
                                start=True, stop=True,
                            )
                        if diag:
                            a_r = a_t.rearrange("p (h q) -> p h q", h=2)
                            s_r = ps_s.rearrange("p (h q) -> p h q", h=2)
                            nc.scalar.activation(a_r[:, :, off:512],
                                                 s_r[:, :, off:512], Exp)
                            for h in (0, 1):
                                if notri:
                                    continue
                                nc.vector.tensor_tensor(
                                    out=a_t[:, 512 * h + off:
                                            512 * h + off + 128],
                                    in0=a_t[:, 512 * h + off:
                                            512 * h + off + 128],
                                    in1=tri_sb[:, 256:384],
                                    op=mybir.AluOpType.mult,
                                )
                        else:
                            nc.scalar.activation(a_t, ps_s, Exp)
                        pend.append((0, jb, off, a_t))
                        pend.append((1, jb, off, a_t))
                    flush_av(4)
                    want = -((-ninj * (jb + 1)) // njb)
                    while ninj - len(inject) < want:
                        inject.pop(0)()
                # drain h0's AV chain first so its normalize overlaps
                # h1's remaining AV matmuls
                recip, rb = {}, {}

                def norm_head(h):
                    recip[h] = wp.tile([1, 512], f32, name="recip", tag="rc",
                                       bufs=4)
                    nc.vector.reciprocal(recip[h], ps_o[h][64:65, :])
                    rb[h] = wp.tile([64, 512], f32, name="rb", tag="rb",
                                    bufs=4)
                    nc.gpsimd.partition_broadcast(rb[h], recip[h])
                    bp = h * 64
                    nc.vector.tensor_tensor(
                        out=oT_sb[t][bp:bp + 64, ib * 512:(ib + 1) * 512],
                        in0=ps_o[h][0:64, :], in1=rb[h],
                        op=mybir.AluOpType.mult,
                    )

                for h in (0, 1):
                    rest = [e for e in pend if e[0] == h]
                    for e in rest:
                        pend.remove(e)
                    for _, jb, off, a_t in rest:
                        nc.tensor.matmul(
                            ps_o[h][:, off:512],
                            lhsT=v1_sb[jb][:, 2 * t + h, :],
                            rhs=a_t[:, 512 * h + off:512 * (h + 1)],
                            start=(jb == 0), stop=(jb == njb - 1),
                            skip_group_check=True,
                        )
                    if norm:
                        norm_head(h)
                while inject:
                    inject.pop(0)()

            def body():
                warm_pe(12)
                ph_dma()
                if stop_after == "dma":
                    return
                if stop_after == "proj":
                    for nb_ in range(4):
                        for t_, kind in ((0, "q"), (0, "k"),
                                         (1, "q"), (1, "k")):
                            qk_group(t_, kind, nb_)()
                        for sc in range(4 * nb_, 4 * nb_ + 4):
                            v_group(sc)()
                    return
                # head: just enough for attn(0, 0)
                qk_group(0, "q", 0)()
                qk_group(0, "k", 0)()
                for sc in range(4):
                    v_group(sc)()
                # injected fillers per attn segment, ordered by DMA
                # arrival and consumer position
                inj = {
                    (0, 0): [qk_group(1, "q", 0), qk_group(1, "k", 0)],
                    (1, 0): [qk_group(0, "q", 1), qk_group(0, "k", 1),
                             v_group(4), v_group(5), v_group(6), v_group(7),
                             qk_group(1, "q", 1), qk_group(1, "k", 1)],
                    (0, 1): [qk_group(0, "q", 2), qk_group(0, "k", 2)],
                    (1, 1): [v_group(8), v_group(9), v_group(10),
                             v_group(11), qk_group(1, "q", 2),
                             qk_group(1, "k", 2),
                             outp_group(0), outp_group(1)],
                    (0, 2): [outp_group(2), outp_group(3),
                             qk_group(0, "q", 3), qk_group(0, "k", 3)],
                    (1, 2): [v_group(12), v_group(13), v_group(14),
                             v_group(15), qk_group(1, "q", 3),
                             qk_group(1, "k", 3),
                             outp_group(4), outp_group(5)],
                    (0, 3): [outp_group(6), outp_group(7),
                             outp_group(8), outp_group(9)],
                    (1, 3): [outp_group(10), outp_group(11)],
                }
                for ib in range(4):
                    for t_ in range(2):
                        attn_block(t_, ib, inj[(t_, ib)])
                for sc in range(12, 16):
                    outp_group(sc, act_evict=True)()

            if reps == 1:
                body()
            else:
                with tc.For_i(0, reps, 1, staggered_reset=stagger,
                              hint_engines=(mybir.EngineType.PE,
                                            mybir.EngineType.Activation,
                                            mybir.EngineType.DVE)):
                    body()

    nc.compile()
    return nc


def _prepare_in_maps(inputs):
    x = np.asarray(inputs["x"], np.float32)
    basin = np.asarray(inputs["basin"], np.float32)
    Wq = np.asarray(inputs["Wq"], np.float32)
    bq = np.asarray(inputs["bq"], np.float32)
    Wk = np.asarray(inputs["Wk"], np.float32)
    bk = np.asarray(inputs["bk"], np.float32)
    Wv = np.asarray(inputs["Wv"], np.float32)
    bv = np.asarray(inputs["bv"], np.float32)
    Wo = np.asarray(inputs["Wo"], np.float32)
    Wb = np.asarray(inputs["Wb"], np.float32)
    bb = np.asarray(inputs["bb"], np.float32)

    gate = 1.0 / (1.0 + np.exp(-(basin @ Wb + bb)))          # [16]
    scale = (gate / np.sqrt(float(DH))).astype(np.float32)   # [16]
    colscale = np.repeat(scale, DH)                          # [1024]
    Wq_s = (Wq * colscale[None, :]).astype(np.float32)
    bq_s = (bq * colscale).astype(np.float32)

    # xt host layout: [q-chunk*128p, kd*512s]: (q*128+p, k*512+s) = xT[k*128+p, q*512+s]
    xt_all = []
    for b in range(B):
        xt_b = x[b].T.astype(np.float16)          # [1024, 2048]
        A = xt_b.reshape(8, 128, 4, 512).transpose(2, 1, 0, 3)
        xt_all.append(np.ascontiguousarray(A.reshape(4 * 128, 8 * 512)))
    eye = np.eye(128, dtype=np.float16)
    msk = (np.tril(np.ones((128, 128), np.float32), -1) *
           -1000.0).astype(np.float16)
    triu = np.triu(np.ones((128, 128), np.float16))
    tri2 = np.ascontiguousarray(np.concatenate([eye, msk, triu], axis=1))

    in_maps = []
    for c in range(N_CORES):
        b, g = divmod(c, 4)
        sl = slice(g * DL, (g + 1) * DL)
        def stack_w(W):
            # [1024, 256] -> [p, kd*256] with element (p, k*256+c) = W[k*128+p, c]
            return np.ascontiguousarray(
                W.reshape(8, 128, DL).transpose(1, 0, 2).reshape(128, 8 * DL)
            ).astype(np.float16)

        in_maps.append({
            "xt": xt_all[b],
            "wq": stack_w(Wq_s[:, sl]),
            "wk": stack_w(Wk[:, sl]),
            "wv": stack_w(Wv[:, sl]),
            "wo": np.ascontiguousarray(
                Wo[sl, :].reshape(2, 128, D).transpose(1, 0, 2)
                .reshape(128, 2 * D)).astype(np.float16),
            "bqk": np.ascontiguousarray(
                np.concatenate([bq_s[sl], bk[sl]]).reshape(4, 128).T),
            "bv": np.ascontiguousarray(bv[sl]).reshape(1, DL),
            "tri": tri2,
        })
    return in_maps


def _run(inputs, trace=False):
    from concourse.bass_utils import run_bass_kernel_spmd

    nc = _CACHE.get("nc")
    if nc is None:
        nc = _build_program()
        _CACHE["nc"] = nc
    in_maps = _prepare_in_maps(inputs)
    res = run_bass_kernel_spmd(nc, in_maps, core_ids=list(range(N_CORES)),
                               trace=trace)
    bo = np.asarray(inputs["bo"], np.float32)
    out = np.zeros((B, S, D), np.float32)
    for c in range(N_CORES):
        out[c // 4] += res.results[c]["out"].astype(np.float32)
    out += bo[None, None, :]
    return out, res


def kernel(**inputs):
    out, _ = _run(inputs, trace=False)
    return out
